# revision 25
# baseline (speedup 1.0000x reference)
"""Causal attention (B=4, S=2048, D=1024, single head) on 8 TRN2 NeuronCores.

Sharding: data-parallel over batch x causal-balanced query split.
  core c -> batch b = c//2, role r = c%2. Role 0 takes the even 128-row
  query tiles, role 1 the odd ones: one SPMD program computing 2p+2 key
  units per slot p is exact for role 1 and wastes one masked unit for
  role 0 (mask is data-driven: qidx input vs kpos iota).

Score trick: scores = (X Wq)(X Wk)^T = X (Wq Wk^T) X^T, so with
  M = Wq Wk^T (batch-independent) the K projection disappears and the
  raw x^T doubles as the key matrix. M is split 640/640 with a 256-row
  overlap: G1 = X M contracts the own 5 a-chunks as cover for the pair
  exchange (the CC mesh cannot complete before ~65us: the framework
  runs two boot barriers to ~51us and the slowest core's arrival adds
  up to ~15us), then G2 adds the partner's 3 chunks (read from
  pair-shared DRAM behind per-jc-half barriers, summed on the DVE).
  V is split by output columns and exchanged behind a third barrier;
  the own half of the PV operand is a local SBUF copy, only the
  partner half is a gated shared-DRAM read.

PE scheduling: HW floor for an N=512 bf16 matmul is ~216ns (1 col @
  2.4GHz + NX overhead) PROVIDED no two consecutive matmuls accumulate
  into the same PSUM bank (same-bank chains serialize at ~259ns).
  Every phase therefore runs 2-4 interleaved accumulation chains:
  M in 2-chain ab-pair passes ordered so the first pass needs only
  1MB of input; V in 4-chain seq-block quads; G in 4 chains (2 et x
  2 sc, the sc pair reusing each weight). Under sustained 8-core load
  the chip can drop to ~2.0GHz (P0), scaling everything by ~1.2x;
  run-to-run spread is dominated by that and by HBM-contended input.

Attention: two-deep software pipeline - qk(i) emits its score matmuls
  (ec-outer, 2-chain key-block groups) and softmax chain; the P^T
  needed by PV comes from ONE blocked DMA-xbar transpose per tile
  (pt[p,u,q] = p[q,u*128+p], a 3D-output dma_start(transpose=True) on
  the scalar queue) replacing all per-unit PE transposes + DVE copies.
  tpv(i) then runs the PV accumulation (lo/hi ctx chains) while
  qk(i-1) and qk(i-2) are already queued, hiding softmax + transpose
  latency; ctx normalization rides the scalar engine (activation Copy
  with per-partition 1/sum scale) so the DVE never blocks the drain.

Head: ~64 cores x 8MB of input contend for chip HBM, so the first
  ~20us are DMA-bound; ~150 warm-up matmuls on a zeroed tile hold the
  HAM clock gate at 8/8 through the ramp. xqt (x^T restricted to own
  query columns, own-chunks-first) is gathered on-chip from xth with
  two strided SBUF->SBUF copies per rank arm instead of being shipped
  again over HBM. Inputs are host-pre-packed to their exact SBUF
  layouts, fat-lined (2-16KB per partition line), and split: sync
  carries the M operands + exchange + out, scalar (ACT) carries
  wkt-jc1/wvh/xth + the transpose stream.

Compute is bf16 with f32 PSUM accumulation; softmax skips the running
max (logits ~N(0,1) after the 1/32 scale; masked lanes sit at -31250
and underflow to exactly 0). Output is written bf16 (the host unshard
upcasts).
"""

import sys

if "/opt/trn_rl_repo" not in sys.path:
    sys.path.insert(0, "/opt/trn_rl_repo")

import ml_dtypes
import numpy as np

import bass_rust

import concourse.bass as bass
import concourse.mybir as mybir
from concourse.tile import TileContext
from concourse.tile_rust import add_dep_helper

B, S, D = 4, 2048, 1024
P = 128
NCORES = 8
DC = D // P           # 8 contraction chunks of 128
QROWS = S // 2        # 1024 query rows per core
QT = QROWS // P       # 8 query tiles of 128 rows
MH = 512              # V column split per rank
MA = 640              # M rows computed per rank (256-row overlap: G1 covers
MAB = MA // P         # 5 of 8 G chunks locally while the pair barrier - which
MCB = (D - MA) // P   # cannot complete before ~65us - delivers the partner's 3
SCALE = 1.0 / np.sqrt(np.float32(D))
MASK_NEG = -1.0e6
GROUPS = [[0, 1], [2, 3], [4, 5], [6, 7]]
N_WARM = 150          # HAM warm-up matmuls while input DMA streams

F32 = mybir.dt.float32
BF16 = mybir.dt.bfloat16


# ---------------------------------------------------------------------------
# This container's walrus build (setupSyncWait, CoreV2/V3GenImpl.cpp) rejects
# any instruction carrying more than one sem wait. Tile's wait-assignment
# freely emits several. Hoist all but one wait of each instruction onto NOPs
# inserted immediately before it on the same engine — the engine executes its
# stream in order, so waiting on a preceding same-engine NOP is equivalent.
def _split_multi_waits(nc):
    n_split = 0
    for fn in nc.m.functions:
        for bb in fn.blocks:
            insts = list(bb.instructions)
            out = []
            changed = False
            for inst in insts:
                si = inst.sync_info
                if si is not None and len(si.on_wait) > 1:
                    waits = list(si.on_wait)
                    for w in waits[:-1]:
                        nop = mybir.InstNoOp(
                            name=f"{inst.name}-wsplit{n_split}", ins=[], outs=[]
                        )
                        n_split += 1
                        nop.engine = inst.engine
                        nop.sync_info = bass_rust.SyncInfo(
                            on_wait=[w], on_update=[]
                        )
                        out.append(nop)
                    inst.sync_info = bass_rust.SyncInfo(
                        on_wait=[waits[-1]], on_update=list(si.on_update)
                    )
                    changed = True
                if si is not None and len(si.on_update) > 2:
                    raise RuntimeError(
                        f"{inst.name}: {len(si.on_update)} sync updates; "
                        "update-splitting not implemented"
                    )
                out.append(inst)
            if changed:
                bb.instructions = out
    return nc
# ---------------------------------------------------------------------------


def _build_nc():
    nc = bass.Bass()

    # All inputs are host-pre-packed to their SBUF layouts (see
    # _shard_inputs): flat contiguous DMAs at max burst size.
    xth = nc.declare_dram_parameter("xth", [2 * P * DC * QROWS], BF16, isOutput=False)
    wqh = nc.declare_dram_parameter("wqh", [MAB * P * DC * P], BF16, isOutput=False)
    wkt = nc.declare_dram_parameter("wkt", [2 * P * DC * 512], BF16, isOutput=False)
    wvh = nc.declare_dram_parameter("wvh", [P * DC * MH], BF16, isOutput=False)
    qidx = nc.declare_dram_parameter("qidx", [QROWS], F32, isOutput=False)
    rk = nc.declare_dram_parameter("rk", [1, 1], mybir.dt.uint32, isOutput=False)
    out = nc.declare_dram_parameter("out", [QROWS, D], BF16, isOutput=True)

    xth_r = xth.rearrange("(sh p dc s) -> sh p dc s", p=P, dc=DC, s=QROWS)
    wqh_r = wqh.rearrange("(ab p ec i) -> ab p ec i", p=P, ec=DC, i=P)
    wkt_r = wkt.rearrange("(jc p ec j) -> jc p ec j", p=P, ec=DC, j=512)
    wvh_r = wvh.rearrange("(p dc e) -> p dc e", p=P, dc=DC, e=MH)
    qidx_r = qidx.rearrange("(t p) -> p t", p=P)

    with TileContext(nc) as tc:
        # The race-detector sim can't model pair-aliased Shared DRAM (it
        # demands a single writer); ordering for the shared exchange is
        # enforced with explicit deps instead.
        tc.race_detector_enabled = False

        persist = tc.alloc_tile_pool(name="persist", bufs=1)
        xth_sb = persist.tile([P, DC, S], BF16, tag="xth_sb")
        qt_sb = persist.tile([P, DC, QROWS], BF16, tag="qt_sb")  # G^T [b, q]
        v_b = [
            persist.tile([P, 512 // P, D], BF16, tag=f"v_b{v}", name=f"v_b{v}")
            for v in range(S // 512)
        ]
        kpos_f = persist.tile([P, S], F32, tag="kpos_f")
        qidx_sb = persist.tile([P, QT], F32, tag="qidx_sb")
        warm_w = persist.tile([P, P], BF16, tag="warm_w")

        nc.sync.dma_start(qidx_sb[:], qidx_r)
        nc.gpsimd.memset(warm_w[:], 0.0)

        # ---- Phase 1: M, V projection, pair exchange, G ----
        with (
            tc.tile_pool(name="proj_in", bufs=1) as proj_in,
            tc.tile_pool(name="proj_st", bufs=1) as proj_st,
            tc.tile_pool(name="proj_ps", bufs=8, space="PSUM") as proj_ps,
            tc.tile_pool(name="cc_dram", bufs=1, space="DRAM") as cc_dram,
        ):
            # HAM warm-up: ~64 back-to-back matmuls on the zeroed tile keep
            # the PE busy while the first input DMAs stream, so the clock
            # gate is at 8/8 when the real matmuls start. Two alternating
            # PSUM slots let consecutive warm MMs overlap.
            warm_ps = [
                proj_ps.tile([P, P], F32, tag="proj_ps", name=f"warm_ps{i}")
                for i in range(2)
            ]
            for i in range(N_WARM):
                nc.tensor.matmul(
                    warm_ps[i % 2][:], warm_w[:], warm_w[:],
                    start=True, stop=True,
                )

            wqh_sb = proj_in.tile([P, MAB * DC, P], BF16, tag="wqh_sb")
            wkt_sb = proj_in.tile([P, 2 * DC, 512], BF16, tag="wkt_sb")
            wvh_sb = proj_in.tile([P, DC, MH], BF16, tag="wvh_sb")
            xqt_sb = proj_in.tile([P, DC, QROWS], BF16, tag="xqt_sb")

            # Input streams, spread over three engine queues in first-use
            # order. sync: the M operands (wqh + wkt, jc0 before jc1).
            # scalar: wvh + xth first half (V starts at seq block 0).
            # gpsimd: xth second half + xqt (needed last, by G).
            # Fat per-partition lines ramp the DMA queues fastest: wqh is
            # 2KB lines, wkt/wvh 8KB, xth/xqt 16KB. M's operands (wqh +
            # wkt jc0) lead the sync queue; wkt jc1 leads scalar so the
            # jc1 pass is never the gate.
            nc.sync.dma_start(wqh_sb[:, 0:DC, :], wqh_r[0])
            nc.sync.dma_start(wqh_sb[:, DC : 2 * DC, :], wqh_r[1])
            nc.sync.dma_start(
                wkt_sb[:, 0 : DC // 2, :], wkt_r[0][:, 0 : DC // 2, :]
            )
            nc.sync.dma_start(
                wkt_sb[:, DC // 2 : DC, :], wkt_r[0][:, DC // 2 : DC, :]
            )
            nc.sync.dma_start(wqh_sb[:, 2 * DC : 3 * DC, :], wqh_r[2])
            nc.sync.dma_start(wqh_sb[:, 3 * DC : 4 * DC, :], wqh_r[3])
            nc.sync.dma_start(wqh_sb[:, 4 * DC : 5 * DC, :], wqh_r[4])
            nc.scalar.dma_start(wkt_sb[:, DC : 2 * DC, :], wkt_r[1])
            nc.scalar.dma_start(wvh_sb[:], wvh_r)
            nc.scalar.dma_start(xth_sb[:, :, 0:QROWS], xth_r[0])
            nc.scalar.dma_start(xth_sb[:, :, QROWS:S], xth_r[1])

            # Emitted late so the (slow) iota doesn't delay anything.
            # iota values < 2048 are exact in f32
            nc.gpsimd.iota(
                kpos_f[:], pattern=[[1, S]], base=0, channel_multiplier=0,
                allow_small_or_imprecise_dtypes=True,
            )

            # ---- M = Wq Wk^T, own MA rows: M[a, b] = sum_e wqh[e,a] wkt[e,b]
            # jc passes sequential (jc0 can start before wkt jc1 lands);
            # within a pass, 4 interleaved chains over the 4 a-blocks, the
            # wkt rhs reused by all 4.
            mst = proj_st.tile([P, MAB, D], BF16, tag="mst")
            m_writes = []
            # 2-chain passes ordered for the DMA trickle: (ab01, jc0) needs
            # only wqh[0:2] + the first wkt half; later passes ride arrivals.
            m_passes = [((0, 1), (0,)), ((0, 1), (1,)), ((2, 3), (0,)),
                        ((2, 3), (1,)), ((4,), (0, 1))]
            for abs_, jcs in m_passes:
                chains = [(ab, jc) for ab in abs_ for jc in jcs]
                pss = [
                    proj_ps.tile([P, 512], F32, tag="proj_ps", name=f"mps{i}")
                    for i in range(len(chains))
                ]
                for ec in range(DC):
                    for i, (ab, jc) in enumerate(chains):
                        nc.tensor.matmul(
                            pss[i][:],
                            wqh_sb[:, ab * DC + ec, :],
                            wkt_sb[:, jc * DC + ec, :],
                            start=(ec == 0),
                            stop=(ec == DC - 1),
                        )
                for i, (ab, jc) in enumerate(chains):
                    nc.scalar.copy(
                        mst[:, ab, jc * 512 : (jc + 1) * 512], pss[i][:]
                    )

            # ---- V[:, own 512 e-cols] for all 2048 rows: 4 chains over
            # seq-block quads, the wvh rhs reused by all 4.
            vst = proj_st.tile([P, S // P, MH], BF16, tag="vst")
            for q4 in range(0, S // P, 4):
                pss = [
                    proj_ps.tile([P, MH], F32, tag="proj_ps", name=f"vps{i}")
                    for i in range(4)
                ]
                for dc in range(DC):
                    for i in range(4):
                        nc.tensor.matmul(
                            pss[i][:],
                            xth_sb[:, dc, (q4 + i) * P : (q4 + i + 1) * P],
                            wvh_sb[:, dc, :],
                            start=(dc == 0),
                            stop=(dc == DC - 1),
                        )
                for i in range(4):
                    nc.scalar.copy(vst[:, q4 + i, :], pss[i][:])

            # One Shared tensor per (rank, slot) — single writer each. V
            # is staged in four 512-row quarter slots so each write can
            # launch as its quarter completes.
            sh_m = [
                [
                    cc_dram.tile(
                        [MA * 512], BF16, tag=f"sh_m{r}{jc}",
                        name=f"sh_m{r}{jc}", addr_space="Shared",
                    )
                    for jc in range(2)
                ]
                for r in range(2)
            ]
            sh_v = [
                [
                    cc_dram.tile(
                        [512 * MH], BF16, tag=f"sh_v{r}{q}",
                        name=f"sh_v{r}{q}", addr_space="Shared",
                    )
                    for q in range(4)
                ]
                for r in range(2)
            ]

            def m_view(flat):
                return flat.rearrange("(ab p b) -> p ab b", p=P, b=512)

            def v_view(flat):
                return flat.rearrange("(sb p e) -> p sb e", p=P, e=MH)

            rk_reg = nc.sync.alloc_register("rk_reg")
            nc.sync.reg_load(rk_reg, rk[0:1, 0:1])

            # Two pair rendezvous: ccm covers M, ccv covers V. Queue order
            # on Sync is critical: the gated m2 read is emitted BEFORE the
            # V writes, so G's partner data does not sit behind a 2MB write
            # stream that itself waits for the V projection to finish.
            # M ships in two jc halves, each behind its own barrier, so the
            # first half of the partner's M is in SBUF well before G's
            # partner-contraction reaches it (G's et 0-3 need only jc0).
            m_writes = [[], []]
            for r in range(2):
                ctx_mgr = (
                    tc.If(nc.sync.snap(rk_reg) == 0) if r == 0 else cmpA.Else()
                )
                with ctx_mgr as branch:
                    if r == 0:
                        cmpA = branch
                    for jc in range(2):
                        m_writes[jc].append(
                            nc.sync.dma_start(
                                m_view(sh_m[r][jc]),
                                mst[:, :, jc * 512 : (jc + 1) * 512],
                            )
                        )
            # xqt (x^T restricted to the own query rows, own-a-chunks
            # first) is gathered on-chip from xth instead of being shipped
            # again over HBM: two strided SBUF->SBUF copies per rank arm
            # (own 5 chunks, then the 3 complement chunks).
            xv = xth_sb[:].rearrange("p d (t two x) -> p d t two x", two=2, x=P)
            xq_v = xqt_sb[:].rearrange("p a (t x) -> p a t x", x=P)
            for r in range(2):
                ctx_mgr = (
                    tc.If(nc.sync.snap(rk_reg) == 0) if r == 0 else cmpX.Else()
                )
                with ctx_mgr as branch:
                    if r == 0:
                        cmpX = branch
                    nc.sync.dma_start(
                        xq_v[:, 0:MAB], xv[:, 3 * r : 3 * r + MAB, :, r, :]
                    )
                    nc.sync.dma_start(
                        xq_v[:, MAB:DC],
                        xv[:, 5 * (1 - r) : 5 * (1 - r) + MCB, :, r, :],
                    )
            ccms = []
            for jc in range(2):
                bm_in = cc_dram.tile([16], F32, tag=f"bm_in{jc}", name=f"bm_in{jc}")
                bm_out = cc_dram.tile(
                    [2, 16], F32, tag=f"bm_out{jc}", name=f"bm_out{jc}"
                )
                ccm = nc.gpsimd.collective_compute(
                    "AllGather",
                    mybir.AluOpType.bypass,
                    replica_groups=GROUPS,
                    ins=[bm_in[:]],
                    outs=[bm_out[:]],
                )
                for w in m_writes[jc]:
                    add_dep_helper(ccm.ins, w.ins, True, "barrier after M writes")
                ccms.append(ccm)

            # Rank 0 needs rows 640:1024 = partner blocks 2:5; rank 1 rows
            # 0:384 = partner blocks 0:3.
            m2_sb = proj_in.tile([P, MCB, D], BF16, tag="m2_sb")
            for r in range(2):
                ctx_mgr = (
                    tc.If(nc.sync.snap(rk_reg) == 0) if r == 0 else cmpB.Else()
                )
                with ctx_mgr as branch:
                    if r == 0:
                        cmpB = branch
                    blo = 2 if r == 0 else 0
                    for jc in range(2):
                        rd = nc.sync.dma_start(
                            m2_sb[:, :, jc * 512 : (jc + 1) * 512],
                            m_view(sh_m[1 - r][jc])[:, blo : blo + MCB, :],
                        )
                        add_dep_helper(rd.ins, ccms[jc].ins, True, "read after rdv")

            # V exchange: ship own columns to the partner, and fill the own
            # half of v_b by local SBUF->SBUF copies (not gated, starts as
            # soon as each vst quarter lands); only the partner half is a
            # gated shared-DRAM read.
            v_writes = []
            for r in range(2):
                ctx_mgr = (
                    tc.If(nc.sync.snap(rk_reg) == 0) if r == 0 else cmpC.Else()
                )
                with ctx_mgr as branch:
                    if r == 0:
                        cmpC = branch
                    for q in range(4):
                        v_writes.append(
                            nc.sync.dma_start(
                                v_view(sh_v[r][q]),
                                vst[:, q * 4 : (q + 1) * 4, :],
                            )
                        )
                        nc.sync.dma_start(
                            v_b[q][:, :, r * MH : (r + 1) * MH],
                            vst[:, q * 4 : (q + 1) * 4, :],
                        )
            bv_in = cc_dram.tile([16], F32, tag="bv_in")
            bv_out = cc_dram.tile([2, 16], F32, tag="bv_out")
            ccv = nc.gpsimd.collective_compute(
                "AllGather",
                mybir.AluOpType.bypass,
                replica_groups=GROUPS,
                ins=[bv_in[:]],
                outs=[bv_out[:]],
            )
            for w in v_writes:
                add_dep_helper(ccv.ins, w.ins, True, "barrier after V writes")

            for r in range(2):
                ctx_mgr = (
                    tc.If(nc.sync.snap(rk_reg) == 0) if r == 0 else cmpD.Else()
                )
                with ctx_mgr as branch:
                    if r == 0:
                        cmpD = branch
                    for kb in range(4):
                        rd = nc.sync.dma_start(
                            v_b[kb][:, :, (1 - r) * MH : (2 - r) * MH],
                            v_view(sh_v[1 - r][kb]),
                        )
                        add_dep_helper(rd.ins, ccv.ins, True, "read after rdv")

            # ---- G^T[b, q] = sum_a M[a, b] x^T[a, q], two stages: G1
            # contracts the own 5 a-chunks (local mst) as barrier cover,
            # G2 the partner's 3 (m2_sb), summed on the DVE. 4 interleaved
            # chains (2 et x 2 sc) per group, same-lhsT sc pairs.
            g1_sb = proj_in.tile([P, DC, QROWS], BF16, tag="g1_sb")
            for e2 in range(0, DC, 2):
                pss = [
                    proj_ps.tile([P, 512], F32, tag="proj_ps", name=f"gps{i}")
                    for i in range(4)
                ]
                for ac in range(MAB):
                    for i in range(2):
                        et = e2 + i
                        for sc in (1, 0):
                            nc.tensor.matmul(
                                pss[2 * i + sc][:],
                                mst[:, ac, et * P : (et + 1) * P],
                                xqt_sb[:, ac, sc * 512 : (sc + 1) * 512],
                                start=(ac == 0),
                                stop=(ac == MAB - 1),
                            )
                for i in range(2):
                    for sc in (1, 0):
                        nc.scalar.copy(
                            g1_sb[:, e2 + i, sc * 512 : (sc + 1) * 512],
                            pss[2 * i + sc][:],
                        )
            for e2 in range(0, DC, 2):
                pss = [
                    proj_ps.tile([P, 512], F32, tag="proj_ps", name=f"hps{i}")
                    for i in range(4)
                ]
                for ac in range(MCB):
                    for i in range(2):
                        et = e2 + i
                        for sc in (1, 0):
                            nc.tensor.matmul(
                                pss[2 * i + sc][:],
                                m2_sb[:, ac, et * P : (et + 1) * P],
                                xqt_sb[:, MAB + ac, sc * 512 : (sc + 1) * 512],
                                start=(ac == 0),
                                stop=(ac == MCB - 1),
                            )
                for sc in (1, 0):
                    for i in range(2):
                        nc.vector.tensor_add(
                            qt_sb[:, e2 + i, sc * 512 : (sc + 1) * 512],
                            pss[2 * i + sc][:],
                            g1_sb[:, e2 + i, sc * 512 : (sc + 1) * 512],
                        )

        # ---- Phase 2: attention, descending tile pairs, software-pipelined
        with (
            tc.tile_pool(name="att", bufs=2) as att,
            tc.tile_pool(name="att_sm", bufs=4) as att_sm,
            tc.tile_pool(name="ps_sc", bufs=4, space="PSUM") as ps_sc,
            tc.tile_pool(name="ps_ctx", bufs=4, space="PSUM") as ps_ctx,
        ):
            def qk(qt):
                nku = 2 * qt + 2
                nkeys = nku * P
                p_sb = att.tile([P, S], BF16, tag="p_sb", bufs=4)
                pt_sb = att.tile([P, S // P, P], BF16, tag="pt_sb", bufs=3)
                sums = att_sm.tile([P, 4], F32, tag="sums")
                qcol = qidx_sb[:, qt : qt + 1]
                blocks = []
                k0 = 0
                while k0 < nkeys:
                    w = min(512, nkeys - k0)
                    blocks.append((k0, w))
                    k0 += w
                vi = 0
                # groups of up to 2 key blocks = 2 interleaved PSUM chains
                # (2 banks: lets two tiles' scores live in the 4-slot pool
                # so the pipeline can run two QK tiles ahead)
                for g0 in range(0, len(blocks), 2):
                    grp = blocks[g0 : g0 + 2]
                    pss = [
                        ps_sc.tile([P, w], F32, tag="sc_ps", name=f"sc_ps{i}")
                        for i, (_, w) in enumerate(grp)
                    ]
                    for ec in range(DC):
                        for ps, (k0, w) in zip(pss, grp):
                            nc.tensor.matmul(
                                ps[:],
                                qt_sb[:, ec, qt * P : (qt + 1) * P],
                                xth_sb[:, ec, k0 : k0 + w],
                                start=(ec == 0),
                                stop=(ec == DC - 1),
                            )
                    for ps, (k0, w) in zip(pss, grp):
                        bias = att_sm.tile([P, w], F32, tag="bias")
                        nc.vector.tensor_scalar(
                            bias[:], kpos_f[:, k0 : k0 + w], qcol, MASK_NEG,
                            mybir.AluOpType.is_gt, mybir.AluOpType.mult,
                        )
                        sm = att_sm.tile([P, w], F32, tag="sm")
                        nc.vector.tensor_add(sm[:], ps[:], bias[:])
                        nc.scalar.activation(
                            p_sb[:, k0 : k0 + w], sm[:],
                            mybir.ActivationFunctionType.Exp,
                            scale=float(SCALE),
                            accum_out=sums[:, vi : vi + 1],
                        )
                        vi += 1
                # One blocked DMA-transpose through the xbar replaces the
                # per-unit PE transposes + DVE copies entirely:
                # pt_sb[p, u, q] = p_sb[q, u*128+p]. Rides the scalar queue
                # right behind this tile's exps; the transfer overlaps the
                # next tile's score matmuls.
                nc.scalar.dma_start(
                    pt_sb[:, 0:nku, :], p_sb[:, 0:nkeys], transpose=True
                )
                return {"qt": qt, "nku": nku, "nblk": vi,
                        "pt_sb": pt_sb, "sums": sums}

            def tpv(st):
                qt, nku = st["qt"], st["nku"]
                pt_sb, sums = st["pt_sb"], st["sums"]

                tot = att_sm.tile([P, 1], F32, tag="tot")
                rinv = att_sm.tile([P, 1], F32, tag="rinv")
                nc.vector.reduce_sum(
                    tot[:], sums[:, : st["nblk"]], axis=mybir.AxisListType.X
                )
                nc.vector.reciprocal(rinv[:], tot[:])

                ctx_lo = ps_ctx.tile([P, 512], F32, tag="ctx", name="ctx_lo")
                ctx_hi = ps_ctx.tile([P, 512], F32, tag="ctx", name="ctx_hi")
                for kc in range(nku):
                    vb = v_b[kc // 4]
                    vrow = kc % 4
                    nc.tensor.matmul(
                        ctx_lo[:], pt_sb[:, kc, :], vb[:, vrow, 0:512],
                        start=(kc == 0), stop=(kc == nku - 1),
                    )
                    nc.tensor.matmul(
                        ctx_hi[:], pt_sb[:, kc, :], vb[:, vrow, 512:D],
                        start=(kc == 0), stop=(kc == nku - 1),
                    )

                out_sb = att.tile([P, D], BF16, tag="out_sb")
                nc.scalar.activation(
                    out_sb[:, 0:512], ctx_lo[:],
                    mybir.ActivationFunctionType.Copy, scale=rinv[:],
                )
                nc.scalar.activation(
                    out_sb[:, 512:D], ctx_hi[:],
                    mybir.ActivationFunctionType.Copy, scale=rinv[:],
                )
                nc.sync.dma_start(out[qt * P : (qt + 1) * P, :], out_sb[:])

            # Two-deep software pipeline: two tiles of score matmuls are
            # always in flight ahead of tpv(i), covering tile i's softmax
            # chain AND its blocked transpose DMA (~2-5us for big tiles).
            sts = [qk(7), qk(6), qk(5)]
            tpv(sts[0])
            for qt in (4, 3, 2, 1, 0):
                sts.append(qk(qt))
                tpv(sts[-3])
            tpv(sts[-2])
            tpv(sts[-1])

        persist.release()

    return _split_multi_waits(nc)


_NC_CACHE = None


def _get_nc():
    global _NC_CACHE
    if _NC_CACHE is None:
        _NC_CACHE = _build_nc()
    return _NC_CACHE


def _qrows(role):
    # 128-row tiles: role 0 -> even tiles, role 1 -> odd tiles.
    return np.concatenate(
        [np.arange((2 * t + role) * P, (2 * t + role + 1) * P) for t in range(QT)]
    )


def _pack_pdc(a, inner):
    """[rows, cols] -> [p, rows//P, cols], rows chunked by P."""
    rows, cols = a.shape
    return np.ascontiguousarray(a.reshape(rows // P, P, cols).transpose(1, 0, 2))


def _shard_inputs(x, Wq, Wk, Wv):
    bf = ml_dtypes.bfloat16
    WqT = Wq.T.astype(bf)                         # [e, a]
    WkT = Wk.T.astype(bf)                         # [e, b]
    Wv_b = Wv.astype(bf)
    in_maps = []
    for c in range(NCORES):
        b, r = c // 2, c % 2
        rows = _qrows(r)
        xbT = x[b].T.astype(bf)                   # [D, S]
        own = slice(0, MA) if r == 0 else slice(D - MA, D)
        # Pre-pack to SBUF layouts (flat DMAs):
        # wqh: [ab, p, ec, 128] from WqT[:, own] [e=1024, a=MA]
        wqh_p = np.ascontiguousarray(
            WqT[:, own].reshape(DC, P, MAB, P).transpose(2, 1, 0, 3)
        )
        # wkt: [jc, p, ec, 512]
        wkt_p = np.ascontiguousarray(
            WkT.reshape(DC, P, 2, 512).transpose(2, 1, 0, 3)
        )
        # wvh: [p, dc, 512] from Wv[:, own 512 cols]
        wvh_p = _pack_pdc(Wv_b[:, r * MH : (r + 1) * MH], MH)
        # xth: [sh, p, dc, 1024] from xbT [D, S]
        xth_p = np.ascontiguousarray(
            xbT.reshape(DC, P, 2, QROWS).transpose(2, 1, 0, 3)
        )

        in_maps.append(
            {
                "xth": xth_p.reshape(-1),
                "wqh": wqh_p.reshape(-1),
                "wkt": wkt_p.reshape(-1),
                "wvh": wvh_p.reshape(-1),
                "qidx": rows.astype(np.float32),
                "rk": np.array([[r]], dtype=np.uint32),
            }
        )
    return in_maps


def _unshard(results, dtype):
    out = np.empty((B, S, D), dtype=dtype)
    for c in range(NCORES):
        b, r = c // 2, c % 2
        out[b, _qrows(r), :] = results[c]["out"].astype(dtype)
    return out


def run(x, Wq, Wk, Wv, trace=False, tmpdir=None):
    from concourse.bass_utils import run_bass_kernel_spmd

    nc = _get_nc()
    in_maps = _shard_inputs(x, Wq, Wk, Wv)
    res = run_bass_kernel_spmd(
        nc, in_maps, core_ids=list(range(NCORES)), trace=trace, tmpdir=tmpdir
    )
    return _unshard(res.results, np.dtype(x.dtype)), res


def kernel(x, Wq, Wk, Wv):
    out, _ = run(np.asarray(x), np.asarray(Wq), np.asarray(Wk), np.asarray(Wv))
    return out


# revision 26
# speedup vs baseline: 1.1760x; 1.1760x over previous
"""Causal attention (B=4, S=2048, D=1024, single head) on 8 TRN2 NeuronCores.

Sharding: data-parallel over batch x causal-balanced query split.
  core c -> batch b = c//2, role r = c%2. Role 0 takes the even 128-row
  query tiles, role 1 the odd ones: one SPMD program computing 2p+2 key
  units per slot p is exact for role 1 and wastes one masked unit for
  role 0 (mask is data-driven: qidx input vs kpos iota).

Score trick: scores = (X Wq)(X Wk)^T = X (Wq Wk^T) X^T, so with
  M = Wq Wk^T (batch-independent) the K projection disappears and the
  raw x^T doubles as the key matrix. M is split 640/640 with a 256-row
  overlap: G1 = X M contracts the own 5 a-chunks as cover for the pair
  exchange (the CC mesh cannot complete before ~65us: the framework
  runs two boot barriers to ~51us and the slowest core's arrival adds
  up to ~15us), then G2 adds the partner's 3 chunks (read from
  pair-shared DRAM behind per-jc-half barriers, summed on the DVE).
  V is split by output columns and exchanged behind a third barrier;
  the own half of the PV operand is a local SBUF copy, only the
  partner half is a gated shared-DRAM read.

PE scheduling: HW floor for an N=512 bf16 matmul is ~216ns (1 col @
  2.4GHz + NX overhead) PROVIDED no two consecutive matmuls accumulate
  into the same PSUM bank (same-bank chains serialize at ~259ns).
  Every phase therefore runs 2-4 interleaved accumulation chains:
  M in 2-chain ab-pair passes ordered so the first pass needs only
  1MB of input; V in 4-chain seq-block quads; G in 4 chains (2 et x
  2 sc, the sc pair reusing each weight). Under sustained 8-core load
  the chip can drop to ~2.0GHz (P0), scaling everything by ~1.2x;
  run-to-run spread is dominated by that and by HBM-contended input.

Attention: two-deep software pipeline - qk(i) emits its score matmuls
  (ec-outer, 2-chain key-block groups) and softmax chain; the P^T
  needed by PV comes from ONE blocked DMA-xbar transpose per tile
  (pt[p,u,q] = p[q,u*128+p], a 3D-output dma_start(transpose=True) on
  the scalar queue) replacing all per-unit PE transposes + DVE copies.
  tpv(i) then runs the PV accumulation (lo/hi ctx chains) while
  qk(i-1) and qk(i-2) are already queued, hiding softmax + transpose
  latency; ctx normalization rides the scalar engine (activation Copy
  with per-partition 1/sum scale) so the DVE never blocks the drain.

Head: ~64 cores x 8MB of input contend for chip HBM, so the first
  ~20us are DMA-bound; ~150 warm-up matmuls on a zeroed tile hold the
  HAM clock gate at 8/8 through the ramp. xqt (x^T restricted to own
  query columns, own-chunks-first) is gathered on-chip from xth with
  two strided SBUF->SBUF copies per rank arm instead of being shipped
  again over HBM. Inputs are host-pre-packed to their exact SBUF
  layouts, fat-lined (2-16KB per partition line), and split: sync
  carries the M operands + exchange + out, scalar (ACT) carries
  wkt-jc1/wvh/xth + the transpose stream.

Compute is bf16 with f32 PSUM accumulation; softmax skips the running
max (logits ~N(0,1) after the 1/32 scale; masked lanes sit at -31250
and underflow to exactly 0). Output is written bf16 (the host unshard
upcasts).
"""

import sys

if "/opt/trn_rl_repo" not in sys.path:
    sys.path.insert(0, "/opt/trn_rl_repo")

import ml_dtypes
import numpy as np

import bass_rust

import concourse.bass as bass
import concourse.mybir as mybir
from concourse.tile import TileContext
from concourse.tile_rust import add_dep_helper

B, S, D = 4, 2048, 1024
P = 128
NCORES = 8
DC = D // P           # 8 contraction chunks of 128
QROWS = S // 2        # 1024 query rows per core
QT = QROWS // P       # 8 query tiles of 128 rows
MH = 512              # V column split per rank
MA = 640              # M rows computed per rank (256-row overlap: G1 covers
MAB = MA // P         # 5 of 8 G chunks locally while the pair barrier - which
MCB = (D - MA) // P   # cannot complete before ~65us - delivers the partner's 3
SCALE = 1.0 / np.sqrt(np.float32(D))
MASK_NEG = -1.0e6
GROUPS = [[0, 1], [2, 3], [4, 5], [6, 7]]
N_WARM = 150          # HAM warm-up matmuls while input DMA streams

F32 = mybir.dt.float32
BF16 = mybir.dt.bfloat16


# ---------------------------------------------------------------------------
# This container's walrus build (setupSyncWait, CoreV2/V3GenImpl.cpp) rejects
# any instruction carrying more than one sem wait. Tile's wait-assignment
# freely emits several. Hoist all but one wait of each instruction onto NOPs
# inserted immediately before it on the same engine — the engine executes its
# stream in order, so waiting on a preceding same-engine NOP is equivalent.
def _split_multi_waits(nc):
    n_split = 0
    for fn in nc.m.functions:
        for bb in fn.blocks:
            insts = list(bb.instructions)
            out = []
            changed = False
            for inst in insts:
                si = inst.sync_info
                if si is not None and len(si.on_wait) > 1:
                    waits = list(si.on_wait)
                    for w in waits[:-1]:
                        nop = mybir.InstNoOp(
                            name=f"{inst.name}-wsplit{n_split}", ins=[], outs=[]
                        )
                        n_split += 1
                        nop.engine = inst.engine
                        nop.sync_info = bass_rust.SyncInfo(
                            on_wait=[w], on_update=[]
                        )
                        out.append(nop)
                    inst.sync_info = bass_rust.SyncInfo(
                        on_wait=[waits[-1]], on_update=list(si.on_update)
                    )
                    changed = True
                if si is not None and len(si.on_update) > 2:
                    raise RuntimeError(
                        f"{inst.name}: {len(si.on_update)} sync updates; "
                        "update-splitting not implemented"
                    )
                out.append(inst)
            if changed:
                bb.instructions = out
    return nc
# ---------------------------------------------------------------------------


def _build_nc():
    nc = bass.Bass()

    # All inputs are host-pre-packed to their SBUF layouts (see
    # _shard_inputs): flat contiguous DMAs at max burst size.
    xth = nc.declare_dram_parameter("xth", [2 * P * DC * QROWS], BF16, isOutput=False)
    wqh = nc.declare_dram_parameter("wqh", [MAB * P * DC * P], BF16, isOutput=False)
    wkt = nc.declare_dram_parameter("wkt", [2 * P * DC * 512], BF16, isOutput=False)
    wvh = nc.declare_dram_parameter("wvh", [P * DC * MH], BF16, isOutput=False)
    qidx = nc.declare_dram_parameter("qidx", [QROWS], F32, isOutput=False)
    rk = nc.declare_dram_parameter("rk", [1, 1], mybir.dt.uint32, isOutput=False)
    out = nc.declare_dram_parameter("out", [QROWS, D], BF16, isOutput=True)

    xth_r = xth.rearrange("(sh p dc s) -> sh p dc s", p=P, dc=DC, s=QROWS)
    wqh_r = wqh.rearrange("(ab p ec i) -> ab p ec i", p=P, ec=DC, i=P)
    wkt_r = wkt.rearrange("(jc p ec j) -> jc p ec j", p=P, ec=DC, j=512)
    wvh_r = wvh.rearrange("(p dc e) -> p dc e", p=P, dc=DC, e=MH)
    qidx_r = qidx.rearrange("(t p) -> p t", p=P)

    with TileContext(nc) as tc:
        # The race-detector sim can't model pair-aliased Shared DRAM (it
        # demands a single writer); ordering for the shared exchange is
        # enforced with explicit deps instead.
        tc.race_detector_enabled = False

        persist = tc.alloc_tile_pool(name="persist", bufs=1)
        xth_sb = persist.tile([P, DC, S], BF16, tag="xth_sb")
        qt_sb = persist.tile([P, DC, QROWS], BF16, tag="qt_sb")  # G^T [b, q]
        v_b = [
            persist.tile([P, 512 // P, D], BF16, tag=f"v_b{v}", name=f"v_b{v}")
            for v in range(S // 512)
        ]
        kpos_f = persist.tile([P, S], F32, tag="kpos_f")
        qidx_sb = persist.tile([P, QT], F32, tag="qidx_sb")
        warm_w = persist.tile([P, P], BF16, tag="warm_w")

        nc.sync.dma_start(qidx_sb[:], qidx_r)
        nc.gpsimd.memset(warm_w[:], 0.0)

        # ---- Phase 1: M, V projection, pair exchange, G ----
        with (
            tc.tile_pool(name="proj_in", bufs=1) as proj_in,
            tc.tile_pool(name="proj_st", bufs=1) as proj_st,
            tc.tile_pool(name="proj_ps", bufs=8, space="PSUM") as proj_ps,
            tc.tile_pool(name="cc_dram", bufs=1, space="DRAM") as cc_dram,
        ):
            # HAM warm-up: ~64 back-to-back matmuls on the zeroed tile keep
            # the PE busy while the first input DMAs stream, so the clock
            # gate is at 8/8 when the real matmuls start. Two alternating
            # PSUM slots let consecutive warm MMs overlap.
            warm_ps = [
                proj_ps.tile([P, P], F32, tag="proj_ps", name=f"warm_ps{i}")
                for i in range(2)
            ]
            for i in range(N_WARM):
                nc.tensor.matmul(
                    warm_ps[i % 2][:], warm_w[:], warm_w[:],
                    start=True, stop=True,
                )

            wqh_sb = proj_in.tile([P, MAB * DC, P], BF16, tag="wqh_sb")
            wkt_sb = proj_in.tile([P, 2 * DC, 512], BF16, tag="wkt_sb")
            wvh_sb = proj_in.tile([P, DC, MH], BF16, tag="wvh_sb")
            xqt_sb = proj_in.tile([P, DC, QROWS], BF16, tag="xqt_sb")

            # Input streams, spread over three engine queues in first-use
            # order. sync: the M operands (wqh + wkt, jc0 before jc1).
            # scalar: wvh + xth first half (V starts at seq block 0).
            # gpsimd: xth second half + xqt (needed last, by G).
            # Fat per-partition lines ramp the DMA queues fastest: wqh is
            # 2KB lines, wkt/wvh 8KB, xth/xqt 16KB. M's operands (wqh +
            # wkt jc0) lead the sync queue; wkt jc1 leads scalar so the
            # jc1 pass is never the gate.
            nc.sync.dma_start(wqh_sb[:, 0:DC, :], wqh_r[0])
            nc.sync.dma_start(wqh_sb[:, DC : 2 * DC, :], wqh_r[1])
            nc.sync.dma_start(
                wkt_sb[:, 0 : DC // 2, :], wkt_r[0][:, 0 : DC // 2, :]
            )
            nc.sync.dma_start(
                wkt_sb[:, DC // 2 : DC, :], wkt_r[0][:, DC // 2 : DC, :]
            )
            nc.sync.dma_start(wqh_sb[:, 2 * DC : 3 * DC, :], wqh_r[2])
            nc.sync.dma_start(wqh_sb[:, 3 * DC : 4 * DC, :], wqh_r[3])
            nc.sync.dma_start(wqh_sb[:, 4 * DC : 5 * DC, :], wqh_r[4])
            nc.scalar.dma_start(wkt_sb[:, DC : 2 * DC, :], wkt_r[1])
            nc.scalar.dma_start(wvh_sb[:], wvh_r)
            nc.scalar.dma_start(xth_sb[:, :, 0:QROWS], xth_r[0])
            nc.scalar.dma_start(xth_sb[:, :, QROWS:S], xth_r[1])

            # Emitted late so the (slow) iota doesn't delay anything.
            # iota values < 2048 are exact in f32
            nc.gpsimd.iota(
                kpos_f[:], pattern=[[1, S]], base=0, channel_multiplier=0,
                allow_small_or_imprecise_dtypes=True,
            )

            # ---- M = Wq Wk^T, own MA rows: M[a, b] = sum_e wqh[e,a] wkt[e,b]
            # jc passes sequential (jc0 can start before wkt jc1 lands);
            # within a pass, 4 interleaved chains over the 4 a-blocks, the
            # wkt rhs reused by all 4.
            mst = proj_st.tile([P, MAB, D], BF16, tag="mst")
            m_writes = []
            # 2-chain passes ordered for the DMA trickle: (ab01, jc0) needs
            # only wqh[0:2] + the first wkt half; later passes ride arrivals.
            m_passes = [((0, 1), (0,)), ((0, 1), (1,)), ((2, 3), (0,)),
                        ((2, 3), (1,)), ((4,), (0, 1))]
            for abs_, jcs in m_passes:
                chains = [(ab, jc) for ab in abs_ for jc in jcs]
                pss = [
                    proj_ps.tile([P, 512], F32, tag="proj_ps", name=f"mps{i}")
                    for i in range(len(chains))
                ]
                for ec in range(DC):
                    for i, (ab, jc) in enumerate(chains):
                        nc.tensor.matmul(
                            pss[i][:],
                            wqh_sb[:, ab * DC + ec, :],
                            wkt_sb[:, jc * DC + ec, :],
                            start=(ec == 0),
                            stop=(ec == DC - 1),
                        )
                for i, (ab, jc) in enumerate(chains):
                    nc.scalar.copy(
                        mst[:, ab, jc * 512 : (jc + 1) * 512], pss[i][:]
                    )

            # ---- V[:, own 512 e-cols] for all 2048 rows: 4 chains over
            # seq-block quads, the wvh rhs reused by all 4.
            vst = proj_st.tile([P, S // P, MH], BF16, tag="vst")
            for q4 in range(0, S // P, 4):
                pss = [
                    proj_ps.tile([P, MH], F32, tag="proj_ps", name=f"vps{i}")
                    for i in range(4)
                ]
                for dc in range(DC):
                    for i in range(4):
                        nc.tensor.matmul(
                            pss[i][:],
                            xth_sb[:, dc, (q4 + i) * P : (q4 + i + 1) * P],
                            wvh_sb[:, dc, :],
                            start=(dc == 0),
                            stop=(dc == DC - 1),
                        )
                for i in range(4):
                    nc.scalar.copy(vst[:, q4 + i, :], pss[i][:])

            # One Shared tensor per (rank, slot) — single writer each. V
            # is staged in four 512-row quarter slots so each write can
            # launch as its quarter completes.
            sh_m = [
                [
                    cc_dram.tile(
                        [MA * 512], BF16, tag=f"sh_m{r}{jc}",
                        name=f"sh_m{r}{jc}", addr_space="Shared",
                    )
                    for jc in range(2)
                ]
                for r in range(2)
            ]
            sh_v = [
                [
                    cc_dram.tile(
                        [512 * MH], BF16, tag=f"sh_v{r}{q}",
                        name=f"sh_v{r}{q}", addr_space="Shared",
                    )
                    for q in range(4)
                ]
                for r in range(2)
            ]

            def m_view(flat):
                return flat.rearrange("(ab p b) -> p ab b", p=P, b=512)

            def v_view(flat):
                return flat.rearrange("(sb p e) -> p sb e", p=P, e=MH)

            rk_reg = nc.sync.alloc_register("rk_reg")
            nc.sync.reg_load(rk_reg, rk[0:1, 0:1])

            # Two pair rendezvous: ccm covers M, ccv covers V. Queue order
            # on Sync is critical: the gated m2 read is emitted BEFORE the
            # V writes, so G's partner data does not sit behind a 2MB write
            # stream that itself waits for the V projection to finish.
            # M ships in two jc halves, each behind its own barrier, so the
            # first half of the partner's M is in SBUF well before G's
            # partner-contraction reaches it (G's et 0-3 need only jc0).
            m_writes = [[], []]
            for r in range(2):
                ctx_mgr = (
                    tc.If(nc.sync.snap(rk_reg) == 0) if r == 0 else cmpA.Else()
                )
                with ctx_mgr as branch:
                    if r == 0:
                        cmpA = branch
                    for jc in range(2):
                        m_writes[jc].append(
                            nc.sync.dma_start(
                                m_view(sh_m[r][jc]),
                                mst[:, :, jc * 512 : (jc + 1) * 512],
                            )
                        )
            # xqt (x^T restricted to the own query rows, own-a-chunks
            # first) is gathered on-chip from xth instead of being shipped
            # again over HBM: two strided SBUF->SBUF copies per rank arm
            # (own 5 chunks, then the 3 complement chunks).
            xv = xth_sb[:].rearrange("p d (t two x) -> p d t two x", two=2, x=P)
            xq_v = xqt_sb[:].rearrange("p a (t x) -> p a t x", x=P)
            for r in range(2):
                ctx_mgr = (
                    tc.If(nc.sync.snap(rk_reg) == 0) if r == 0 else cmpX.Else()
                )
                with ctx_mgr as branch:
                    if r == 0:
                        cmpX = branch
                    nc.sync.dma_start(
                        xq_v[:, 0:MAB], xv[:, 3 * r : 3 * r + MAB, :, r, :]
                    )
                    nc.sync.dma_start(
                        xq_v[:, MAB:DC],
                        xv[:, 5 * (1 - r) : 5 * (1 - r) + MCB, :, r, :],
                    )
            ccms = []
            for jc in range(2):
                bm_in = cc_dram.tile([16], F32, tag=f"bm_in{jc}", name=f"bm_in{jc}")
                bm_out = cc_dram.tile(
                    [2, 16], F32, tag=f"bm_out{jc}", name=f"bm_out{jc}"
                )
                ccm = nc.gpsimd.collective_compute(
                    "AllGather",
                    mybir.AluOpType.bypass,
                    replica_groups=GROUPS,
                    ins=[bm_in[:]],
                    outs=[bm_out[:]],
                )
                for w in m_writes[jc]:
                    add_dep_helper(ccm.ins, w.ins, True, "barrier after M writes")
                ccms.append(ccm)

            # Rank 0 needs rows 640:1024 = partner blocks 2:5; rank 1 rows
            # 0:384 = partner blocks 0:3.
            m2_sb = proj_in.tile([P, MCB, D], BF16, tag="m2_sb")
            for r in range(2):
                ctx_mgr = (
                    tc.If(nc.sync.snap(rk_reg) == 0) if r == 0 else cmpB.Else()
                )
                with ctx_mgr as branch:
                    if r == 0:
                        cmpB = branch
                    blo = 2 if r == 0 else 0
                    for jc in range(2):
                        rd = nc.sync.dma_start(
                            m2_sb[:, :, jc * 512 : (jc + 1) * 512],
                            m_view(sh_m[1 - r][jc])[:, blo : blo + MCB, :],
                        )
                        add_dep_helper(rd.ins, ccms[jc].ins, True, "read after rdv")

            # V exchange: ship own columns to the partner, and fill the own
            # half of v_b by local SBUF->SBUF copies (not gated, starts as
            # soon as each vst quarter lands); only the partner half is a
            # gated shared-DRAM read.
            v_writes = []
            for r in range(2):
                ctx_mgr = (
                    tc.If(nc.sync.snap(rk_reg) == 0) if r == 0 else cmpC.Else()
                )
                with ctx_mgr as branch:
                    if r == 0:
                        cmpC = branch
                    for q in range(4):
                        v_writes.append(
                            nc.sync.dma_start(
                                v_view(sh_v[r][q]),
                                vst[:, q * 4 : (q + 1) * 4, :],
                            )
                        )
                        nc.sync.dma_start(
                            v_b[q][:, :, r * MH : (r + 1) * MH],
                            vst[:, q * 4 : (q + 1) * 4, :],
                        )
            bv_in = cc_dram.tile([16], F32, tag="bv_in")
            bv_out = cc_dram.tile([2, 16], F32, tag="bv_out")
            ccv = nc.gpsimd.collective_compute(
                "AllGather",
                mybir.AluOpType.bypass,
                replica_groups=GROUPS,
                ins=[bv_in[:]],
                outs=[bv_out[:]],
            )
            for w in v_writes:
                add_dep_helper(ccv.ins, w.ins, True, "barrier after V writes")

            for r in range(2):
                ctx_mgr = (
                    tc.If(nc.sync.snap(rk_reg) == 0) if r == 0 else cmpD.Else()
                )
                with ctx_mgr as branch:
                    if r == 0:
                        cmpD = branch
                    for kb in range(4):
                        rd = nc.sync.dma_start(
                            v_b[kb][:, :, (1 - r) * MH : (2 - r) * MH],
                            v_view(sh_v[1 - r][kb]),
                        )
                        add_dep_helper(rd.ins, ccv.ins, True, "read after rdv")

            # ---- G^T[b, q] = sum_a M[a, b] x^T[a, q], two stages: G1
            # contracts the own 5 a-chunks (local mst) as barrier cover,
            # G2 the partner's 3 (m2_sb), summed on the DVE. 4 interleaved
            # chains (2 et x 2 sc) per group, same-lhsT sc pairs.
            g1_sb = proj_in.tile([P, DC, QROWS], BF16, tag="g1_sb")
            for e2 in range(0, DC, 2):
                pss = [
                    proj_ps.tile([P, 512], F32, tag="proj_ps", name=f"gps{i}")
                    for i in range(4)
                ]
                for ac in range(MAB):
                    for i in range(2):
                        et = e2 + i
                        for sc in (1, 0):
                            nc.tensor.matmul(
                                pss[2 * i + sc][:],
                                mst[:, ac, et * P : (et + 1) * P],
                                xqt_sb[:, ac, sc * 512 : (sc + 1) * 512],
                                start=(ac == 0),
                                stop=(ac == MAB - 1),
                            )
                for i in range(2):
                    for sc in (1, 0):
                        nc.scalar.copy(
                            g1_sb[:, e2 + i, sc * 512 : (sc + 1) * 512],
                            pss[2 * i + sc][:],
                        )
            for e2 in range(0, DC, 2):
                pss = [
                    proj_ps.tile([P, 512], F32, tag="proj_ps", name=f"hps{i}")
                    for i in range(4)
                ]
                for ac in range(MCB):
                    last = ac == MCB - 1
                    # final round: both sc=1 closers first, so the DVE sums
                    # that gate the first attention tile overlap the tail
                    # of the matmul stream
                    order = (
                        [(i, sc) for i in range(2) for sc in (1, 0)]
                        if not last
                        else [(0, 1), (1, 1), (0, 0), (1, 0)]
                    )
                    for i, sc in order:
                        et = e2 + i
                        nc.tensor.matmul(
                            pss[2 * i + sc][:],
                            m2_sb[:, ac, et * P : (et + 1) * P],
                            xqt_sb[:, MAB + ac, sc * 512 : (sc + 1) * 512],
                            start=(ac == 0),
                            stop=last,
                        )
                for sc in (1, 0):
                    for i in range(2):
                        nc.vector.tensor_add(
                            qt_sb[:, e2 + i, sc * 512 : (sc + 1) * 512],
                            pss[2 * i + sc][:],
                            g1_sb[:, e2 + i, sc * 512 : (sc + 1) * 512],
                        )

        # ---- Phase 2: attention, descending tile pairs, software-pipelined
        with (
            tc.tile_pool(name="att", bufs=2) as att,
            tc.tile_pool(name="att_sm", bufs=4) as att_sm,
            tc.tile_pool(name="ps_sc", bufs=4, space="PSUM") as ps_sc,
            tc.tile_pool(name="ps_ctx", bufs=4, space="PSUM") as ps_ctx,
        ):
            def qk(qt):
                nku = 2 * qt + 2
                nkeys = nku * P
                p_sb = att.tile([P, S], BF16, tag="p_sb", bufs=4)
                pt_sb = att.tile([P, S // P, P], BF16, tag="pt_sb", bufs=3)
                sums = att_sm.tile([P, 4], F32, tag="sums")
                qcol = qidx_sb[:, qt : qt + 1]
                blocks = []
                k0 = 0
                while k0 < nkeys:
                    w = min(512, nkeys - k0)
                    blocks.append((k0, w))
                    k0 += w
                vi = 0
                # groups of up to 2 key blocks = 2 interleaved PSUM chains
                # (2 banks: lets two tiles' scores live in the 4-slot pool
                # so the pipeline can run two QK tiles ahead)
                for g0 in range(0, len(blocks), 2):
                    grp = blocks[g0 : g0 + 2]
                    pss = [
                        ps_sc.tile([P, w], F32, tag="sc_ps", name=f"sc_ps{i}")
                        for i, (_, w) in enumerate(grp)
                    ]
                    for ec in range(DC):
                        for ps, (k0, w) in zip(pss, grp):
                            nc.tensor.matmul(
                                ps[:],
                                qt_sb[:, ec, qt * P : (qt + 1) * P],
                                xth_sb[:, ec, k0 : k0 + w],
                                start=(ec == 0),
                                stop=(ec == DC - 1),
                            )
                    for ps, (k0, w) in zip(pss, grp):
                        bias = att_sm.tile([P, w], F32, tag="bias")
                        nc.vector.tensor_scalar(
                            bias[:], kpos_f[:, k0 : k0 + w], qcol, MASK_NEG,
                            mybir.AluOpType.is_gt, mybir.AluOpType.mult,
                        )
                        sm = att_sm.tile([P, w], F32, tag="sm")
                        nc.vector.tensor_add(sm[:], ps[:], bias[:])
                        nc.scalar.activation(
                            p_sb[:, k0 : k0 + w], sm[:],
                            mybir.ActivationFunctionType.Exp,
                            scale=float(SCALE),
                            accum_out=sums[:, vi : vi + 1],
                        )
                        vi += 1
                # One blocked DMA-transpose through the xbar replaces the
                # per-unit PE transposes + DVE copies entirely:
                # pt_sb[p, u, q] = p_sb[q, u*128+p]. Rides the scalar queue
                # right behind this tile's exps; the transfer overlaps the
                # next tile's score matmuls.
                nc.scalar.dma_start(
                    pt_sb[:, 0:nku, :], p_sb[:, 0:nkeys], transpose=True
                )
                return {"qt": qt, "nku": nku, "nblk": vi,
                        "pt_sb": pt_sb, "sums": sums}

            def tpv(st):
                qt, nku = st["qt"], st["nku"]
                pt_sb, sums = st["pt_sb"], st["sums"]

                tot = att_sm.tile([P, 1], F32, tag="tot")
                rinv = att_sm.tile([P, 1], F32, tag="rinv")
                nc.vector.reduce_sum(
                    tot[:], sums[:, : st["nblk"]], axis=mybir.AxisListType.X
                )
                nc.vector.reciprocal(rinv[:], tot[:])

                ctx_lo = ps_ctx.tile([P, 512], F32, tag="ctx", name="ctx_lo")
                ctx_hi = ps_ctx.tile([P, 512], F32, tag="ctx", name="ctx_hi")
                for kc in range(nku):
                    vb = v_b[kc // 4]
                    vrow = kc % 4
                    nc.tensor.matmul(
                        ctx_lo[:], pt_sb[:, kc, :], vb[:, vrow, 0:512],
                        start=(kc == 0), stop=(kc == nku - 1),
                    )
                    nc.tensor.matmul(
                        ctx_hi[:], pt_sb[:, kc, :], vb[:, vrow, 512:D],
                        start=(kc == 0), stop=(kc == nku - 1),
                    )

                out_sb = att.tile([P, D], BF16, tag="out_sb")
                nc.scalar.activation(
                    out_sb[:, 0:512], ctx_lo[:],
                    mybir.ActivationFunctionType.Copy, scale=rinv[:],
                )
                nc.sync.dma_start(
                    out[qt * P : (qt + 1) * P, 0:512], out_sb[:, 0:512]
                )
                nc.scalar.activation(
                    out_sb[:, 512:D], ctx_hi[:],
                    mybir.ActivationFunctionType.Copy, scale=rinv[:],
                )
                nc.sync.dma_start(
                    out[qt * P : (qt + 1) * P, 512:D], out_sb[:, 512:D]
                )

            # Two-deep software pipeline: two tiles of score matmuls are
            # always in flight ahead of tpv(i), covering tile i's softmax
            # chain AND its blocked transpose DMA (~2-5us for big tiles).
            sts = [qk(7), qk(6), qk(5)]
            tpv(sts[0])
            for qt in (4, 3, 2, 1, 0):
                sts.append(qk(qt))
                tpv(sts[-3])
            tpv(sts[-2])
            tpv(sts[-1])

        persist.release()

    return _split_multi_waits(nc)


_NC_CACHE = None


def _get_nc():
    global _NC_CACHE
    if _NC_CACHE is None:
        _NC_CACHE = _build_nc()
    return _NC_CACHE


def _qrows(role):
    # 128-row tiles: role 0 -> even tiles, role 1 -> odd tiles.
    return np.concatenate(
        [np.arange((2 * t + role) * P, (2 * t + role + 1) * P) for t in range(QT)]
    )


def _pack_pdc(a, inner):
    """[rows, cols] -> [p, rows//P, cols], rows chunked by P."""
    rows, cols = a.shape
    return np.ascontiguousarray(a.reshape(rows // P, P, cols).transpose(1, 0, 2))


def _shard_inputs(x, Wq, Wk, Wv):
    bf = ml_dtypes.bfloat16
    WqT = Wq.T.astype(bf)                         # [e, a]
    WkT = Wk.T.astype(bf)                         # [e, b]
    Wv_b = Wv.astype(bf)
    in_maps = []
    for c in range(NCORES):
        b, r = c // 2, c % 2
        rows = _qrows(r)
        xbT = x[b].T.astype(bf)                   # [D, S]
        own = slice(0, MA) if r == 0 else slice(D - MA, D)
        # Pre-pack to SBUF layouts (flat DMAs):
        # wqh: [ab, p, ec, 128] from WqT[:, own] [e=1024, a=MA]
        wqh_p = np.ascontiguousarray(
            WqT[:, own].reshape(DC, P, MAB, P).transpose(2, 1, 0, 3)
        )
        # wkt: [jc, p, ec, 512]
        wkt_p = np.ascontiguousarray(
            WkT.reshape(DC, P, 2, 512).transpose(2, 1, 0, 3)
        )
        # wvh: [p, dc, 512] from Wv[:, own 512 cols]
        wvh_p = _pack_pdc(Wv_b[:, r * MH : (r + 1) * MH], MH)
        # xth: [sh, p, dc, 1024] from xbT [D, S]
        xth_p = np.ascontiguousarray(
            xbT.reshape(DC, P, 2, QROWS).transpose(2, 1, 0, 3)
        )

        in_maps.append(
            {
                "xth": xth_p.reshape(-1),
                "wqh": wqh_p.reshape(-1),
                "wkt": wkt_p.reshape(-1),
                "wvh": wvh_p.reshape(-1),
                "qidx": rows.astype(np.float32),
                "rk": np.array([[r]], dtype=np.uint32),
            }
        )
    return in_maps


def _unshard(results, dtype):
    out = np.empty((B, S, D), dtype=dtype)
    for c in range(NCORES):
        b, r = c // 2, c % 2
        out[b, _qrows(r), :] = results[c]["out"].astype(dtype)
    return out


def run(x, Wq, Wk, Wv, trace=False, tmpdir=None):
    from concourse.bass_utils import run_bass_kernel_spmd

    nc = _get_nc()
    in_maps = _shard_inputs(x, Wq, Wk, Wv)
    res = run_bass_kernel_spmd(
        nc, in_maps, core_ids=list(range(NCORES)), trace=trace, tmpdir=tmpdir
    )
    return _unshard(res.results, np.dtype(x.dtype)), res


def kernel(x, Wq, Wk, Wv):
    out, _ = run(np.asarray(x), np.asarray(Wq), np.asarray(Wk), np.asarray(Wv))
    return out
